# revision 8
# baseline (speedup 1.0000x reference)
"""NetVLAD Trainium2 kernel (Bass/Tile), data-parallel over batch on 8 cores.

Problem shapes (hardcoded): x [32, 512, 40, 40] f32, centroids/conv_w [64, 512],
conv_b [64].  Output: [32, 32768] f32.

Precision strategy: x, w, b are split on the host into fp16 hi/lo pairs
(x = xh + xl exactly up to 2^-22 rel).  Scores use three fp16 matmuls
(xh*wh + xl*wh + xh*wl) accumulated in fp32 PSUM -> fp32-grade scores at
1 cyc/row PE throughput (fp32 matmuls cost 4 cyc/row).  The VLAD aggregation
runs on xh only (2^-11 rel inputs), which after intra-normalization leaves
~1e-4 relative output error.  Total HBM traffic is unchanged (2 x fp16 = fp32).

Per-core program (4 items each):
  scores[p,k] = b + sum_c x[c,p] w[k,c]   (PE; x blocks stationary, b via ones-row)
  soft = softmax_k(scores)                 (DVE max/recip + ACT exp, PSUM-direct)
  x_pc = xh^T per 128x128 block            (PE transpose -> PSUM -> SBUF copy)
  agg[k, 1+c] += soft_chunk^T @ [1 | x_pc] (PE, accumulated over 13 p-chunks;
                                            col 0 gives mass_k)
  vlad = (agg - mass*cent), intra-L2-norm over c, global L2 norm -> out
"""

import numpy as np

N, C, HW, K = 32, 512, 1600, 64
NCORES = 8
IPC = N // NCORES          # items per core
CB = C // 128              # channel blocks (4)
NP = (HW + 127) // 128     # pixel chunks per item (13; last is 64 wide)

AGG_USE_LO = False         # add the soft @ xl correction matmul (2x agg cost)

_CACHE = {}
LAST_RESULTS = None


def _build():
    import contextlib
    import concourse.bacc as bacc
    import concourse.mybir as mybir
    import concourse.tile as tile
    from concourse.masks import make_identity
    import concourse.bass as bass

    dt = mybir.dt
    f32 = dt.float32
    f16 = dt.float16

    nc = bacc.Bacc(None, target_bir_lowering=False, debug=False)

    xh_d = nc.dram_tensor("xh", [IPC, C, HW], f16, kind="ExternalInput").ap()
    xl_d = nc.dram_tensor("xl", [IPC, C, HW], f16, kind="ExternalInput").ap()
    wh_d = nc.dram_tensor("wh", [C, K], f16, kind="ExternalInput").ap()
    wl_d = nc.dram_tensor("wl", [C, K], f16, kind="ExternalInput").ap()
    bh_d = nc.dram_tensor("bh", [K], f16, kind="ExternalInput").ap()
    bl_d = nc.dram_tensor("bl", [K], f16, kind="ExternalInput").ap()
    cent_d = nc.dram_tensor("cent", [K, C], f32, kind="ExternalInput").ap()
    out_d = nc.dram_tensor("out", [IPC, K, C], f32, kind="ExternalOutput").ap()

    with tile.TileContext(nc) as tc:
        ctx = contextlib.ExitStack()
        with ctx:
            singles = ctx.enter_context(tc.tile_pool(name="singles", bufs=1))
            xin = ctx.enter_context(tc.tile_pool(name="xin", bufs=2))
            xpc = ctx.enter_context(tc.tile_pool(name="xpc", bufs=4))
            sm = ctx.enter_context(tc.tile_pool(name="sm", bufs=4))
            small = ctx.enter_context(tc.tile_pool(name="small", bufs=8))
            epi = ctx.enter_context(tc.tile_pool(name="epi", bufs=2))
            ps_s = ctx.enter_context(tc.tile_pool(name="ps_s", bufs=2, space="PSUM"))
            ps_t = ctx.enter_context(tc.tile_pool(name="ps_t", bufs=2, space="PSUM"))
            ps_a = ctx.enter_context(tc.tile_pool(name="ps_a", bufs=1, space="PSUM"))
            ps_g = ctx.enter_context(tc.tile_pool(name="ps_g", bufs=1, space="PSUM"))

            # ---- constants ----
            def load_w(d, tag):
                t = singles.tile([128, CB, K], f16, tag=tag)
                nc.sync.dma_start(out=t, in_=d.rearrange("(cb c) k -> c cb k", c=128))
                return t
            wh_sb, wl_sb = load_w(wh_d, "wh"), load_w(wl_d, "wl")

            def load_b(d, tag):
                t = singles.tile([1, K], f16, tag=tag)
                nc.sync.dma_start(
                    out=t, in_=bass.AP(tensor=d.tensor, offset=d.offset,
                                       ap=[[0, 1]] + list(d.ap)))
                return t
            bh_sb, bl_sb = load_b(bh_d, "bh"), load_b(bl_d, "bl")

            ones_row = singles.tile([1, 128], f16)
            nc.vector.memset(ones_row, 1.0)
            cent_sb = singles.tile([K, C], f32)
            nc.sync.dma_start(out=cent_sb, in_=cent_d)
            ident = singles.tile([128, 128], f16)
            make_identity(nc, ident)
            ones64 = singles.tile([K, 1], f32)
            nc.vector.memset(ones64, 1.0)
            ones1x64 = singles.tile([1, K], f32)
            nc.vector.memset(ones1x64, 1.0)

            # accumulated per-item results for the batched epilogue
            nv_all = singles.tile([K, IPC, C], f32)      # mass*cent - agg
            mass_all = singles.tile([K, IPC], f32)

            for n in range(IPC):
                xh_sb = xin.tile([128, CB, HW], f16, tag="xh")
                nc.sync.dma_start(
                    out=xh_sb, in_=xh_d[n].rearrange("(cb c) p -> c cb p", c=128))
                xl_sb = xin.tile([128, CB, HW], f16, tag="xl")
                nc.sync.dma_start(
                    out=xl_sb, in_=xl_d[n].rearrange("(cb c) p -> c cb p", c=128))

                aggA = ps_a.tile([K, 257], f32, tag="aggA")
                aggB = ps_a.tile([K, 256], f32, tag="aggB")

                for pc in range(NP):
                    p0 = pc * 128
                    pw = min(128, HW - p0)

                    # --- scores+bias in fp16 hi/lo splits -> fp32 PSUM ---
                    ps = ps_s.tile([128, K], f32, tag="scores")
                    nc.tensor.matmul(ps[:pw], lhsT=ones_row[:, :pw], rhs=bh_sb,
                                     start=True, stop=False)
                    nc.tensor.matmul(ps[:pw], lhsT=ones_row[:, :pw], rhs=bl_sb,
                                     start=False, stop=False)
                    for cb in range(CB):
                        xh_blk = xh_sb[:, cb, p0:p0 + pw]
                        xl_blk = xl_sb[:, cb, p0:p0 + pw]
                        nc.tensor.matmul(ps[:pw], lhsT=xh_blk, rhs=wh_sb[:, cb],
                                         start=False, stop=False)
                        nc.tensor.matmul(ps[:pw], lhsT=xh_blk, rhs=wl_sb[:, cb],
                                         start=False, stop=False)
                        nc.tensor.matmul(ps[:pw], lhsT=xl_blk, rhs=wh_sb[:, cb],
                                         start=False, stop=(cb == CB - 1))

                    # --- softmax over free dim (K), reading PSUM directly ---
                    negmax = small.tile([128, 1], f32, tag="negmax")
                    nc.vector.reduce_max(out=negmax[:pw], in_=ps[:pw],
                                         axis=mybir.AxisListType.X, negate=True)
                    soft = sm.tile([128, K], f16, tag="soft")
                    sums = small.tile([128, 1], f32, tag="sums")
                    nc.scalar.activation(
                        out=soft[:pw], in_=ps[:pw],
                        func=mybir.ActivationFunctionType.Exp,
                        bias=negmax[:pw], scale=1.0, accum_out=sums[:pw])
                    recip = small.tile([128, 1], f32, tag="recip")
                    nc.vector.reciprocal(out=recip[:pw], in_=sums[:pw])
                    nc.vector.tensor_scalar_mul(soft[:pw], soft[:pw], recip[:pw])

                    # --- transpose xh block: [pw, C] ---
                    ps_x = ps_t.tile([128, C], f16, tag="xt")
                    for cb in range(CB):
                        nc.tensor.transpose(
                            ps_x[:pw, cb * 128:(cb + 1) * 128],
                            xh_sb[:, cb, p0:p0 + pw], ident)
                    x_p = xpc.tile([128, 1 + C], f16, tag="x_p")
                    nc.vector.memset(x_p[:pw, 0:1], 1.0)
                    half = C // 2
                    nc.vector.tensor_copy(out=x_p[:pw, 1:1 + half], in_=ps_x[:pw, :half])
                    nc.scalar.copy(out=x_p[:pw, 1 + half:1 + C], in_=ps_x[:pw, half:])

                    # --- agg matmuls (accumulate over pc) ---
                    nc.tensor.matmul(aggA, lhsT=soft[:pw], rhs=x_p[:pw, 0:257],
                                     start=(pc == 0), stop=(pc == NP - 1))
                    nc.tensor.matmul(aggB, lhsT=soft[:pw], rhs=x_p[:pw, 257:513],
                                     start=(pc == 0), stop=(pc == NP - 1))

                # --- per-item: nv = mass*cent - agg  (minus-vlad; sign folded later)
                nc.vector.tensor_copy(out=mass_all[:, n:n + 1], in_=aggA[:, 0:1])
                nc.vector.scalar_tensor_tensor(
                    out=nv_all[:, n, 0:256], in0=cent_sb[:, 0:256],
                    scalar=mass_all[:, n:n + 1], in1=aggA[:, 1:257],
                    op0=mybir.AluOpType.mult, op1=mybir.AluOpType.subtract)
                nc.vector.scalar_tensor_tensor(
                    out=nv_all[:, n, 256:512], in0=cent_sb[:, 256:512],
                    scalar=mass_all[:, n:n + 1], in1=aggB,
                    op0=mybir.AluOpType.mult, op1=mybir.AluOpType.subtract)

            # ---- batched epilogue over all IPC items ----
            vsq = epi.tile([K, IPC * C], f32, tag="vsq")
            flat_nv = nv_all.rearrange("k i c -> k (i c)")
            nc.vector.tensor_mul(vsq, flat_nv, flat_nv)
            ssq = epi.tile([K, IPC], f32, tag="ssq")
            nc.vector.tensor_reduce(
                out=ssq, in_=vsq.rearrange("k (i c) -> k i c", i=IPC),
                axis=mybir.AxisListType.X, op=mybir.AluOpType.add)
            nrm = epi.tile([K, IPC], f32, tag="nrm")
            nc.scalar.sqrt(nrm, ssq)
            nc.vector.tensor_scalar_max(nrm, nrm, 1e-12)
            inv = epi.tile([K, IPC], f32, tag="inv")
            nc.vector.reciprocal(out=inv, in_=nrm)
            inv2 = epi.tile([K, IPC], f32, tag="inv2")
            nc.vector.tensor_mul(inv2, inv, inv)
            ssq2 = epi.tile([K, IPC], f32, tag="ssq2")
            nc.vector.tensor_mul(ssq2, ssq, inv2)
            # global sumsq per item: [1, IPC] = ones64^T @ ssq2
            g_ps = ps_g.tile([1, IPC], f32, tag="g")
            nc.tensor.matmul(g_ps, lhsT=ones64, rhs=ssq2, start=True, stop=True)
            g_sb = epi.tile([1, IPC], f32, tag="g_sb")
            nc.scalar.sqrt(g_sb, g_ps)
            nc.vector.tensor_scalar_max(g_sb, g_sb, 1e-12)
            ginv = epi.tile([1, IPC], f32, tag="ginv")
            nc.vector.reciprocal(out=ginv, in_=g_sb)
            gb_ps = ps_g.tile([K, IPC], f32, tag="gb")
            nc.tensor.matmul(gb_ps, lhsT=ones1x64, rhs=ginv, start=True, stop=True)
            # scale_k = -(inv * ginv)  (minus compensates nv sign)
            scale_k = epi.tile([K, IPC], f32, tag="scale_k")
            nc.vector.scalar_tensor_tensor(
                out=scale_k, in0=inv, scalar=-1.0, in1=gb_ps,
                op0=mybir.AluOpType.mult, op1=mybir.AluOpType.mult)
            ostage = singles.tile([K, IPC, C], f32)
            for n in range(IPC):
                nc.vector.tensor_scalar_mul(
                    ostage[:, n], nv_all[:, n], scale_k[:, n:n + 1])
            nc.sync.dma_start(
                out=out_d.rearrange("i k c -> k i c"), in_=ostage)

    nc.compile()
    return nc


def _get_nc():
    key = ("f16split", AGG_USE_LO)
    if key not in _CACHE:
        _CACHE[key] = _build()
    return _CACHE[key]


def _split16(a):
    hi = a.astype(np.float16)
    lo = (a - hi.astype(np.float32)).astype(np.float16)
    return hi, lo


def kernel(x, centroids, conv_w, conv_b, _trace=False, **trace_kwargs):
    global LAST_RESULTS
    from concourse import bass_utils

    x = np.ascontiguousarray(np.asarray(x, dtype=np.float32)).reshape(N, C, HW)
    centroids = np.asarray(centroids, dtype=np.float32)
    conv_w = np.asarray(conv_w, dtype=np.float32)
    conv_b = np.asarray(conv_b, dtype=np.float32)

    xh, xl = _split16(x)
    wh, wl = _split16(np.ascontiguousarray(conv_w.T))
    bh, bl = _split16(conv_b)

    nc = _get_nc()
    in_maps = []
    for c in range(NCORES):
        in_maps.append({
            "xh": xh[c * IPC:(c + 1) * IPC],
            "xl": xl[c * IPC:(c + 1) * IPC],
            "wh": wh, "wl": wl, "bh": bh, "bl": bl,
            "cent": centroids,
        })
    res = bass_utils.run_bass_kernel_spmd(
        nc, in_maps, core_ids=list(range(NCORES)), trace=_trace, **trace_kwargs)
    LAST_RESULTS = res
    out = np.concatenate([res.results[c]["out"].reshape(IPC, K * C)
                          for c in range(NCORES)], axis=0)
    return out
